# revision 1
# baseline (speedup 1.0000x reference)
"""Trainium2 Bass kernel for nn_Block_3539053052091 (hedgehog-style linear
attention block with ALiBi-decay mask, smeared keys, and sandwich layernorms).

Strategy (8 NeuronCores):
  - heads are sharded: core c owns heads {2c, 2c+1} for both batches.
  - per core: x is replicated; the core computes LN(x) once per 128-row tile,
    transposes it, and projects with head-sliced, LN-folded weights
    (v, p, q, k all at once).
  - the causal decayed attention  tril(q k^T * gamma^(i-j))  is computed as a
    chunked linear attention (chunk = 128 rows): an intra-chunk masked matmul
    plus a decayed running state S (d x (d+1), with an appended ones-column
    for the normalizer row-sums).
  - each core produces its partial z = (silu(p) * o) @ W_out[rows]; a
    ReduceScatter sums partials and hands each core 512 rows, which it
    layernorms and outputs; the host concatenates the 8 slices.
"""

import numpy as np

import concourse.bass as bass
import concourse.mybir as mybir
import concourse.tile as tile
from concourse import bacc
from concourse.masks import make_identity

f32 = mybir.dt.float32
f32r = mybir.dt.float32r

N_CORES = 8
B = 2
L = 2048
D_MODEL = 1024
HEADS = 16
EXP = 2
D_EXP = D_MODEL * EXP          # 2048
D_HEAD = D_EXP // HEADS        # 128
HPC = HEADS // N_CORES         # heads per core = 2
C = 128                        # chunk (= row tile) size
ROWS = B * L                   # 4096 flattened rows
NT = ROWS // C                 # 32 row tiles
TPB = L // C                   # 16 tiles per batch
KT = D_MODEL // 128            # 8 contraction tiles
LN_EPS = 1e-5
ATTN_EPS = 1e-5

Act = mybir.ActivationFunctionType
Alu = mybir.AluOpType


def build_kernel(mm_dt: str = "f32", reps: int = 1, no_collective: bool = False):
    """Build the single-core SPMD program. mm_dt in {"f32", "f32r"} selects
    the dtype of the big projection matmuls (f32r = TF32-like, 4x faster)."""
    use_r = mm_dt == "f32r"
    wdt = f32r if use_r else f32
    NKT = D_EXP // 128             # 16 k-tiles for the output projection

    nc = bacc.Bacc("TRN2", target_bir_lowering=False, debug=False,
                   num_devices=N_CORES)

    x_in = nc.dram_tensor("x", [ROWS, D_MODEL], f32, kind="ExternalInput")
    xt_in = nc.dram_tensor("xt", [D_MODEL, ROWS], f32, kind="ExternalInput")
    wvp_in = nc.dram_tensor("wvp", [D_MODEL, 4 * D_HEAD], f32, kind="ExternalInput")
    wq_in = nc.dram_tensor("wq", [D_MODEL, 4 * D_HEAD], f32, kind="ExternalInput")
    fvp_in = nc.dram_tensor("fvp", [2, 4 * D_HEAD], f32, kind="ExternalInput")
    fq_in = nc.dram_tensor("fq", [2, 4 * D_HEAD], f32, kind="ExternalInput")
    wout_in = nc.dram_tensor("wout", [D_EXP, D_MODEL], f32, kind="ExternalInput")
    outw_in = nc.dram_tensor("outw", [D_MODEL], f32, kind="ExternalInput")
    outb_in = nc.dram_tensor("outb", [D_MODEL], f32, kind="ExternalInput")
    dt_in = nc.dram_tensor("dtmask", [HPC, C, C], f32, kind="ExternalInput")
    lam_in = nc.dram_tensor("lam", [HPC, C], f32, kind="ExternalInput")
    mus_in = nc.dram_tensor("mus", [HPC, C], f32, kind="ExternalInput")
    sig_in = nc.dram_tensor("sig", [HPC, C], f32, kind="ExternalInput")
    omsig_in = nc.dram_tensor("omsig", [HPC, C], f32, kind="ExternalInput")
    sinv_in = nc.dram_tensor("sinv", [HPC, C], f32, kind="ExternalInput")
    gamc_in = nc.dram_tensor("gamc", [HPC, C], f32, kind="ExternalInput")

    out_ext = nc.dram_tensor("out", [ROWS // N_CORES, D_MODEL], f32,
                             kind="ExternalOutput")
    RB = ROWS // N_CORES  # 512 rows per core after the exchange
    nex = 2 if reps > 1 else 1
    pot_dram = nc.dram_tensor("pot", [nex, N_CORES, HPC * D_HEAD, RB], f32)
    potex_dram = nc.dram_tensor("potex", [nex, N_CORES, HPC * D_HEAD, RB], f32)

    def bcast_ap(handle, parts=128):
        ap = handle.ap()
        return bass.AP(tensor=ap.tensor, offset=ap.offset,
                       ap=[[0, parts]] + list(ap.ap))

    xt_ap = xt_in.ap().rearrange("(kt p) r -> p kt r", p=128)
    if use_r:
        xt_ap = xt_ap.bitcast(f32r)

    with tile.TileContext(nc) as tc:
        with (
            tc.tile_pool(name="const", bufs=1) as cst,
            tc.tile_pool(name="xp", bufs=2) as xp,
            tc.tile_pool(name="zrp", bufs=1) as zrp,
            tc.tile_pool(name="work", bufs=2) as wk,
            tc.tile_pool(name="small", bufs=4) as sm,
            tc.tile_pool(name="state", bufs=2) as st,
            tc.tile_pool(name="statp", bufs=1) as sp,
            tc.tile_pool(name="pt", bufs=1, space="PSUM") as pt,
            tc.tile_pool(name="pproj", bufs=3, space="PSUM") as pproj,
            tc.tile_pool(name="po", bufs=2, space="PSUM") as pO,
            tc.tile_pool(name="psm", bufs=2, space="PSUM") as psm,
        ):
            # ---- constants ----
            ident = cst.tile([128, 128], f32)
            make_identity(nc, ident[:])
            eps_t = cst.tile([128, 1], f32)
            nc.vector.memset(eps_t[:], LN_EPS)

            wvp_sb = cst.tile([128, KT, 4 * D_HEAD], wdt)
            wq_sb = cst.tile([128, KT, 4 * D_HEAD], wdt)
            wout_sb = cst.tile([128, NKT, D_MODEL], wdt)
            for dst, src in ((wvp_sb, wvp_in), (wq_sb, wq_in),
                             (wout_sb, wout_in)):
                ap = src.ap().rearrange("(kt p) n -> p kt n", p=128)
                if use_r:
                    ap = ap.bitcast(f32r)
                nc.sync.dma_start(out=dst, in_=ap)

            fvp_sb = cst.tile([2, 4 * D_HEAD], wdt)
            fq_sb = cst.tile([2, 4 * D_HEAD], wdt)
            for dst, src in ((fvp_sb, fvp_in), (fq_sb, fq_in)):
                ap = src.ap()
                if use_r:
                    ap = ap.bitcast(f32r)
                nc.sync.dma_start(out=dst, in_=ap)

            dt_sb = cst.tile([128, HPC, C], f32)
            nc.sync.dma_start(out=dt_sb, in_=dt_in.ap().rearrange("h b a -> b h a"))
            pv = {}
            for name, src in (("lam", lam_in), ("mus", mus_in), ("sig", sig_in),
                              ("omsig", omsig_in), ("sinv", sinv_in),
                              ("gamc", gamc_in)):
                t = cst.tile([128, HPC], f32, name=f"pv_{name}", tag=f"pv_{name}")
                nc.sync.dma_start(out=t, in_=src.ap().rearrange("h p -> p h"))
                pv[name] = t

            outw_bc = cst.tile([128, D_MODEL], f32)
            outb_bc = cst.tile([128, D_MODEL], f32)
            nc.sync.dma_start(out=outw_bc, in_=bcast_ap(outw_in))
            nc.sync.dma_start(out=outb_bc, in_=bcast_ap(outb_in))

            for rep in range(reps):
                # ---- stats prepass over all row tiles (batched ACT) ----
                mv_all = sp.tile([128, NT, 2], f32, tag="mv_all")
                for t in range(NT):
                    x_t = xp.tile([128, D_MODEL], f32, tag="x")
                    nc.gpsimd.dma_start(out=x_t, in_=x_in[t * C:(t + 1) * C, :])
                    stats = sm.tile([128, 2, 6], f32, tag="stats")
                    for i in range(2):
                        nc.vector.bn_stats(out=stats[:, i, :],
                                           in_=x_t[:, i * 512:(i + 1) * 512])
                    nc.vector.bn_aggr(out=mv_all[:, t, :], in_=stats[:])
                GS = 8
                ln_all = sp.tile([128, NT], f32, tag="ln_all")
                rstd_all = sp.tile([128, NT], f32, tag="rstd_all")
                nrstd_all = sp.tile([128, NT], f32, tag="nrstd_all")
                msn_all = sp.tile([128, NT, 2], f32, tag="msn_all")
                for g in range(0, NT, GS):
                    gs = slice(g, g + GS)
                    nc.scalar.activation(out=ln_all[:, gs],
                                         in_=mv_all[:, gs, 1],
                                         func=Act.Ln, bias=eps_t[:])
                    nc.scalar.activation(out=rstd_all[:, gs], in_=ln_all[:, gs],
                                         func=Act.Exp, scale=-0.5)
                    nc.vector.tensor_scalar_mul(out=nrstd_all[:, gs],
                                                in0=rstd_all[:, gs],
                                                scalar1=-1.0)
                    nc.vector.tensor_scalar_mul(out=msn_all[:, gs, 0],
                                                in0=mv_all[:, gs, 0],
                                                scalar1=-1.0)
                    nc.scalar.activation(out=msn_all[:, gs, 1],
                                         in_=ln_all[:, gs],
                                         func=Act.Exp, scale=0.5)

                S_old = [None, None]
                carry = None
                for t in range(NT):
                    chunk = t % TPB
                    if chunk == 0:
                        for h in range(HPC):
                            S_old[h] = st.tile([128, D_HEAD + 1], f32,
                                               tag=f"S{h}", name=f"S_init{h}")
                            nc.vector.memset(S_old[h][:], 0.0)
                        carry = st.tile([1, 2 * D_HEAD], f32, tag="carry")
                        nc.vector.memset(carry[:], 0.0)
                    rstd = rstd_all[:, t:t + 1]
                    nrstd = nrstd_all[:, t:t + 1]
                    ms_ps = psm.tile([2, 128], f32, tag="sm")
                    nc.tensor.transpose(ms_ps[:], msn_all[:, t, :], ident[:])
                    mustd_t = sm.tile([2, 128], wdt, tag="mustd")
                    nc.vector.tensor_copy(out=mustd_t[:], in_=ms_ps[:])
                    mustd = mustd_t[:]

                    # ---- raw-x projections with rank-2 LN/bias fixup ----
                    xT = wk.tile([128, KT, 128], wdt, tag="xT")
                    nc.sync.dma_start(out=xT, in_=xt_ap[:, :, t * C:(t + 1) * C])
                    ps_vp = pproj.tile([128, 4 * D_HEAD], f32, tag="proj")
                    ps_qk = pproj.tile([128, 4 * D_HEAD], f32, tag="proj")
                    ps_q = ps_qk[:, 0:2 * D_HEAD]
                    ps_k = ps_qk[:, 2 * D_HEAD:4 * D_HEAD]
                    for ps, w_sb, f_sb in ((ps_vp, wvp_sb, fvp_sb),
                                           (ps_qk, wq_sb, fq_sb)):
                        for k in range(KT):
                            nc.tensor.matmul(ps[:], xT[:, k, :], w_sb[:, k, :],
                                             start=(k == 0), stop=False)
                        nc.tensor.matmul(ps[:], mustd, f_sb[:],
                                         start=False, stop=True)

                    # ---- v_aug (rstd fold) and silu(p) ----
                    v_aug = [None, None]
                    for h in range(HPC):
                        v_aug[h] = wk.tile([128, D_HEAD + 1], f32,
                                           tag=f"vaug{h}", name=f"vaug{h}")
                        nc.vector.tensor_scalar_mul(
                            out=v_aug[h][:, 0:D_HEAD],
                            in0=ps_vp[:, h * D_HEAD:(h + 1) * D_HEAD],
                            scalar1=rstd)
                        nc.vector.memset(v_aug[h][:, D_HEAD:D_HEAD + 1], 1.0)
                    # silu(p) = p * (1/(1+exp(-p))), p = rstd * ps_p
                    p_psum = ps_vp[:, 2 * D_HEAD:4 * D_HEAD]
                    emp = wk.tile([128, 2 * D_HEAD], f32, tag="emp")
                    nc.scalar.activation(out=emp[:], in_=p_psum, func=Act.Exp,
                                         scale=nrstd)
                    nc.vector.tensor_scalar_add(out=emp[:], in0=emp[:], scalar1=1.0)
                    sig_p = wk.tile([128, 2 * D_HEAD], f32, tag="sigp")
                    nc.vector.reciprocal(out=sig_p[:], in_=emp[:])
                    silu_p = wk.tile([128, 2 * D_HEAD], f32, tag="silup")
                    nc.vector.scalar_tensor_tensor(
                        out=silu_p[:], in0=p_psum, scalar=rstd,
                        in1=sig_p[:], op0=Alu.mult, op1=Alu.mult)

                    # ---- q/k feature maps (exp with rstd fold + Z accum) ----
                    expq = wk.tile([128, 2 * D_HEAD], f32, tag="expq")
                    zq = sm.tile([128, HPC], f32, tag="zq")
                    expk = wk.tile([128, 2 * D_HEAD], f32, tag="expk")
                    zk = sm.tile([128, HPC], f32, tag="zk")
                    for h in range(HPC):
                        hs = slice(h * D_HEAD, (h + 1) * D_HEAD)
                        nc.scalar.activation(out=expq[:, hs], in_=ps_q[:, hs],
                                             func=Act.Exp, scale=rstd,
                                             accum_out=zq[:, h:h + 1])
                        nc.scalar.activation(out=expk[:, hs], in_=ps_k[:, hs],
                                             func=Act.Exp, scale=rstd,
                                             accum_out=zk[:, h:h + 1])
                    qhat = wk.tile([128, 2 * D_HEAD], f32, tag="qhat")
                    khat = wk.tile([128, 2 * D_HEAD], f32, tag="khat")
                    for h in range(HPC):
                        hs = slice(h * D_HEAD, (h + 1) * D_HEAD)
                        rz = sm.tile([128, 1], f32, tag="rzq")
                        nc.vector.reciprocal(out=rz[:], in_=zq[:, h:h + 1])
                        nc.vector.tensor_scalar(
                            out=qhat[:, hs], in0=expq[:, hs],
                            scalar1=rz[:], scalar2=pv["sinv"][:, h:h + 1],
                            op0=Alu.mult, op1=Alu.mult)
                        rzk = sm.tile([128, 1], f32, tag="rzk")
                        nc.vector.reciprocal(out=rzk[:], in_=zk[:, h:h + 1])
                        nc.vector.tensor_scalar(
                            out=khat[:, hs], in0=expk[:, hs],
                            scalar1=rzk[:], scalar2=pv["sinv"][:, h:h + 1],
                            op0=Alu.mult, op1=Alu.mult)

                    # ---- smear ----
                    kprev = wk.tile([128, 2 * D_HEAD], f32, tag="kprev")
                    nc.scalar.dma_start(out=kprev[0:1, :], in_=carry[0:1, :])
                    nc.scalar.dma_start(out=kprev[1:128, :], in_=khat[0:127, :])
                    carry_new = st.tile([1, 2 * D_HEAD], f32, tag="carry")
                    nc.scalar.dma_start(out=carry_new[:], in_=khat[127:128, :])
                    carry = carry_new
                    ktil = wk.tile([128, 2 * D_HEAD], f32, tag="ktil")
                    kmu = wk.tile([128, 2 * D_HEAD], f32, tag="kmu")
                    for h in range(HPC):
                        hs = slice(h * D_HEAD, (h + 1) * D_HEAD)
                        nc.vector.tensor_scalar_mul(
                            out=kprev[:, hs], in0=kprev[:, hs],
                            scalar1=pv["sig"][:, h:h + 1])
                        nc.vector.scalar_tensor_tensor(
                            out=ktil[:, hs], in0=khat[:, hs],
                            scalar=pv["omsig"][:, h:h + 1], in1=kprev[:, hs],
                            op0=Alu.mult, op1=Alu.add)
                        nc.vector.tensor_scalar_mul(
                            out=kmu[:, hs], in0=ktil[:, hs],
                            scalar1=pv["mus"][:, h:h + 1])

                    # ---- transposes of qhat, ktil ----
                    qT = wk.tile([128, HPC, 128], f32, tag="qT")
                    kT = wk.tile([128, HPC, 128], f32, tag="kT")
                    for h in range(HPC):
                        hs = slice(h * D_HEAD, (h + 1) * D_HEAD)
                        tp = pt.tile([128, 128], f32, tag="tp")
                        nc.tensor.transpose(tp[:], qhat[:, hs], ident[:])
                        nc.vector.tensor_copy(out=qT[:, h, :], in_=tp[:])
                        tp2 = pt.tile([128, 128], f32, tag="tp")
                        nc.tensor.transpose(tp2[:], ktil[:, hs], ident[:])
                        nc.vector.tensor_copy(out=kT[:, h, :], in_=tp2[:])

                    # ---- attention per head ----
                    po = wk.tile([128, 2 * D_HEAD], f32, tag="po")
                    for h in range(HPC):
                        hs = slice(h * D_HEAD, (h + 1) * D_HEAD)
                        at_ps = psm.tile([128, 128], f32, tag="sm")
                        nc.tensor.matmul(at_ps[:], kT[:, h, :], qT[:, h, :],
                                         start=True, stop=True)
                        atm = wk.tile([128, 128], f32, tag="atm")
                        nc.vector.tensor_mul(atm[:], at_ps[:], dt_sb[:, h, :])
                        o1 = pO.tile([128, D_HEAD + 1], f32, tag="O")
                        nc.tensor.matmul(o1[:], atm[:], v_aug[h][:],
                                         start=True, stop=True)
                        o2 = pO.tile([128, D_HEAD + 1], f32, tag="O")
                        nc.tensor.matmul(o2[:], qT[:, h, :], S_old[h][:],
                                         start=True, stop=True)
                        o_sb = wk.tile([128, D_HEAD + 1], f32, tag="osb")
                        nc.vector.tensor_scalar_mul(out=o_sb[:], in0=o2[:],
                                                    scalar1=pv["lam"][:, h:h + 1])
                        nc.vector.tensor_add(o_sb[:], o_sb[:], o1[:])
                        den = sm.tile([128, 1], f32, tag="den")
                        nc.vector.tensor_scalar_add(
                            out=den[:], in0=o_sb[:, D_HEAD:D_HEAD + 1],
                            scalar1=ATTN_EPS)
                        rden = sm.tile([128, 1], f32, tag="rden")
                        nc.vector.reciprocal(out=rden[:], in_=den[:])
                        nc.vector.scalar_tensor_tensor(
                            out=po[:, hs], in0=o_sb[:, 0:D_HEAD],
                            scalar=rden[:], in1=silu_p[:, hs],
                            op0=Alu.mult, op1=Alu.mult)
                        # state update
                        s_ps = psm.tile([128, D_HEAD + 1], f32, tag="sm")
                        nc.tensor.matmul(s_ps[:], kmu[:, hs], v_aug[h][:],
                                         start=True, stop=True)
                        s_new = st.tile([128, D_HEAD + 1], f32, tag=f"S{h}",
                                        name=f"S_new{h}")
                        nc.vector.scalar_tensor_tensor(
                            out=s_new[:], in0=S_old[h][:],
                            scalar=pv["gamc"][:, h:h + 1], in1=s_ps[:],
                            op0=Alu.mult, op1=Alu.add)
                        S_old[h] = s_new

                    # ---- transpose po and ship to the exchange buffer ----
                    rb, cs = t // (RB // C), t % (RB // C)
                    poT = wk.tile([128, HPC, 128], f32, tag="poT")
                    for h in range(HPC):
                        hs = slice(h * D_HEAD, (h + 1) * D_HEAD)
                        tp = psm.tile([128, 128], f32, tag="sm")
                        nc.tensor.transpose(tp[:], po[:, hs], ident[:])
                        nc.vector.tensor_copy(out=poT[:, h, :], in_=tp[:])
                    nc.scalar.dma_start(
                        out=pot_dram[rep % nex, rb].rearrange(
                            "(h p) r -> p h r", p=128)[:, :, cs * C:(cs + 1) * C],
                        in_=poT[:])

                # ---- all-to-all row/col exchange + out proj + final LN ----
                pex = potex_dram[rep % nex]
                pin = pot_dram[rep % nex]
                if no_collective:
                    nc.sync.dma_start(out=pex, in_=pin)
                else:
                    nc.gpsimd.collective_compute(
                        "AllToAll", Alu.bypass,
                        replica_groups=[list(range(N_CORES))],
                        ins=[pin], outs=[pex])
                potex_flat = pex.rearrange(
                    "s d r -> (s d) r").rearrange("(kt p) r -> p kt r", p=128)
                if use_r:
                    potex_flat = potex_flat.bitcast(f32r)
                zts = []
                mvf = sp.tile([128, RB // C, 2], f32, tag="mvf")
                for t in range(RB // C):
                    pox = wk.tile([128, NKT, 128], wdt, tag="pox")
                    nc.sync.dma_start(out=pox,
                                      in_=potex_flat[:, :, t * C:(t + 1) * C])
                    zr_t = zrp.tile([128, D_MODEL], f32, tag=f"zr{t}",
                                    name=f"zr{t}")
                    for n in range(2):
                        ns = slice(n * 512, (n + 1) * 512)
                        z_ps = pproj.tile([128, 512], f32, tag="proj")
                        for kt in range(NKT):
                            nc.tensor.matmul(z_ps[:], pox[:, kt, :],
                                             wout_sb[:, kt, ns],
                                             start=(kt == 0),
                                             stop=(kt == NKT - 1))
                        nc.vector.tensor_copy(out=zr_t[:, ns], in_=z_ps[:])
                    zts.append(zr_t)
                    stats = sm.tile([128, 2, 6], f32, tag="stats")
                    for i in range(2):
                        nc.vector.bn_stats(out=stats[:, i, :],
                                           in_=zr_t[:, i * 512:(i + 1) * 512])
                    nc.vector.bn_aggr(out=mvf[:, t, :], in_=stats[:])
                lnf = sp.tile([128, RB // C], f32, tag="lnf")
                nc.scalar.activation(out=lnf[:], in_=mvf[:, :, 1],
                                     func=Act.Ln, bias=eps_t[:])
                rstdf = sp.tile([128, RB // C], f32, tag="rstdf")
                nc.scalar.activation(out=rstdf[:], in_=lnf[:],
                                     func=Act.Exp, scale=-0.5)
                for t in range(RB // C):
                    o_t = xp.tile([128, D_MODEL], f32, tag="y")
                    nc.vector.tensor_scalar(
                        out=o_t[:], in0=zts[t][:], scalar1=mvf[:, t, 0:1],
                        scalar2=rstdf[:, t:t + 1], op0=Alu.subtract,
                        op1=Alu.mult)
                    nc.vector.tensor_mul(o_t[:], o_t[:], outw_bc[:])
                    nc.vector.tensor_add(o_t[:], o_t[:], outb_bc[:])
                    nc.sync.dma_start(out=out_ext[t * C:(t + 1) * C, :], in_=o_t[:])

    nc.compile()
    return nc


def prepare_in_maps(inputs: dict):
    """Host-side: fold LN affine params into weights, slice per core, build
    per-head decay constants."""
    x = np.ascontiguousarray(np.asarray(inputs["x"], np.float32)
                             .reshape(ROWS, D_MODEL))
    xt = np.ascontiguousarray(x.T)
    W_in = np.asarray(inputs["W_in"], np.float32)
    W_out = np.asarray(inputs["W_out"], np.float32)
    Wq = np.asarray(inputs["Wq"], np.float32)
    Wk = np.asarray(inputs["Wk"], np.float32)
    bq = np.asarray(inputs["bq"], np.float32)
    bk = np.asarray(inputs["bk"], np.float32)
    in_w = np.asarray(inputs["in_ln_w"], np.float32)
    in_b = np.asarray(inputs["in_ln_b"], np.float32)
    q_w = np.asarray(inputs["q_ln_w"], np.float32)
    q_b = np.asarray(inputs["q_ln_b"], np.float32)
    k_w = np.asarray(inputs["k_ln_w"], np.float32)
    k_b = np.asarray(inputs["k_ln_b"], np.float32)
    outw = np.asarray(inputs["out_ln_w"], np.float32)
    outb = np.asarray(inputs["out_ln_b"], np.float32)
    smear = np.asarray(inputs["smear_factor"], np.float32)
    log_scale = np.asarray(inputs["log_scale"], np.float32)

    Wvp_f = W_in * in_w[:, None]
    bvp_f = in_b @ W_in
    Wq_f = Wq * q_w[:, None]
    bq_f = bq + q_b @ Wq
    Wk_f = Wk * k_w[:, None]
    bk_f = bk + k_b @ Wk

    h2 = HEADS // 2
    slopes = np.concatenate([2.0 ** np.linspace(0.0, -8.0, h2),
                             np.zeros(HEADS - h2)]).astype(np.float64)
    sigm = 1.0 / (1.0 + np.exp(-smear.astype(np.float64)))
    s = np.exp(log_scale.astype(np.float64))

    a = np.arange(C)
    diff = a[:, None] - a[None, :]          # i - j
    in_maps = []
    for c in range(N_CORES):
        heads = [HPC * c + i for i in range(HPC)]
        vcols = np.concatenate(
            [np.arange(h * D_HEAD, (h + 1) * D_HEAD) for h in heads])
        pcols = vcols + D_EXP
        dts, lams, muss, sigs, omsigs, sinvs, gamcs = [], [], [], [], [], [], []
        for h in heads:
            lg = -slopes[h]                  # log gamma
            D = np.where(diff >= 0, np.exp(lg * diff), 0.0)   # [i, j]
            dts.append(D.T.astype(np.float32))                # [j, i] = [b, a]
            lams.append(np.exp(lg * (a + 1)).astype(np.float32))
            muss.append(np.exp(lg * (C - 1 - a)).astype(np.float32))
            sigs.append(np.full(C, sigm[h], np.float32))
            omsigs.append(np.full(C, 1.0 - sigm[h], np.float32))
            sinvs.append(np.full(C, 1.0 / s[h], np.float32))
            gamcs.append(np.full(C, np.exp(lg * C), np.float32))
        wvp_c = np.ascontiguousarray(
            np.concatenate([Wvp_f[:, vcols], Wvp_f[:, pcols]], axis=1))
        bvp_c = np.concatenate([bvp_f[vcols], bvp_f[pcols]])
        wq_c = np.ascontiguousarray(Wq_f[:, vcols])
        wk_c = np.ascontiguousarray(Wk_f[:, vcols])
        in_maps.append({
            "x": x,
            "xt": xt,
            "wvp": wvp_c,
            "fvp": np.ascontiguousarray(
                np.stack([wvp_c.sum(0), bvp_c]).astype(np.float32)),
            "wq": np.ascontiguousarray(np.concatenate([wq_c, wk_c], axis=1)),
            "fq": np.ascontiguousarray(np.stack([
                np.concatenate([wq_c.sum(0), wk_c.sum(0)]),
                np.concatenate([bq_f[vcols], bk_f[vcols]])]).astype(np.float32)),
            "wout": W_out,
            "outw": outw, "outb": outb,
            "dtmask": np.stack(dts),
            "lam": np.stack(lams),
            "mus": np.stack(muss),
            "sig": np.stack(sigs),
            "omsig": np.stack(omsigs),
            "sinv": np.stack(sinvs),
            "gamc": np.stack(gamcs),
        })
    return in_maps


DEFAULT_MM_DT = "f32"

_CACHED = {}


def _get_runner(mm_dt=None, reps=1):
    if mm_dt is None:
        mm_dt = DEFAULT_MM_DT
    key = (mm_dt, reps)
    if key not in _CACHED:
        from concourse.bass_utils import run_bass_kernel_spmd  # noqa
        nc = build_kernel(mm_dt=mm_dt, reps=reps)
        _CACHED[key] = nc
    return _CACHED[key]


def kernel(**inputs) -> np.ndarray:
    nc = _get_runner()
    in_maps = prepare_in_maps(inputs)
    from concourse.bass_utils import run_bass_kernel_spmd
    res = run_bass_kernel_spmd(nc, in_maps, list(range(N_CORES)))
    out = np.concatenate([res.results[c]["out"] for c in range(N_CORES)], axis=0)
    return out.reshape(B, L, D_MODEL)

